# revision 3
# baseline (speedup 1.0000x reference)
"""Trainium2 Bass kernel for capsule-network dynamic routing.

Problem: u [64, 2048, 16], W [2048, 16, 1024] ->
  uhat = einsum('bni,nij->bnj', u, W)  (viewed [B, N, 32, 32])
  3 routing iterations (softmax over out-caps, squash) -> v [64, 32, 32]

Sharding: n (input capsules) split across 8 cores, 256 per core.
W slice stays SBUF-resident; uhat is recomputed on the PE each routing
pass (never materialized to HBM).  The per-iteration s-reduction
([64, 32, 32] partial sums) is AllReduced across cores.

Layouts prepared host-side per core (n_local = 256, q = n//8, p8 = n%8):
  WB [32, 128, 1024]: WB[q, 16*p8+i, j] = W[q*8+p8, i, j]
  uB [128, 2048]:     uB[16*p8+i, q*64+b] = u[b, q*8+p8, i]        (pass A)
  uZ [128, 4096]:     r=p8//2, h=p8%2:
                      uZ[32r+16h+i, (2q+h)*64+b] = u[b, n, i], 0 elsewhere
"""

import numpy as np

B = 64
N_FULL = 2048
D_IN = 16
N_OUT = 32
D_OUT = 32
J = N_OUT * D_OUT  # 1024
N_CORES = 8
NL = N_FULL // N_CORES  # 256 local capsules
QB = NL // 8  # 32 q-blocks

_CACHE = {}


def _pack_inputs(u, W):
    """Shard along n and build per-core SBUF-friendly layouts."""
    in_maps = []
    for c in range(N_CORES):
        ul = u[:, c * NL:(c + 1) * NL, :]          # [64, 256, 16]
        Wl = W[c * NL:(c + 1) * NL]                # [256, 16, 1024]
        # WB[q, 16*p8+i, j]
        WB = np.ascontiguousarray(
            Wl.reshape(QB, 8, D_IN, J).reshape(QB, 128, J))
        # uB[16*p8+i, q*64+b] = u[b, q*8+p8, i]
        uB = np.ascontiguousarray(
            ul.reshape(B, QB, 8, D_IN).transpose(2, 3, 1, 0).reshape(128, QB * B))
        # uZ[32r+16h+i, (2q+h)*64+b]
        uZ = np.zeros((4, 32, 2 * QB, B), dtype=np.float32)  # [r, 32row, col/64, b]
        un = ul.reshape(B, QB, 4, 2, D_IN)  # [b, q, r, h, i]
        for h in range(2):
            # rows 16h..16h+16, cols (2q+h)*64
            uZ[:, 16 * h:16 * h + 16, h::2, :] = un[:, :, :, h, :].transpose(2, 3, 1, 0)
        uZ = uZ.reshape(4 * 32, 2 * QB * B)
        in_maps.append({"WB": WB, "uB": uB, "uZ": uZ})
    return in_maps


def _build_program():
    import concourse.bass as bass
    import concourse.tile as tile
    from concourse import bacc, mybir

    f32 = mybir.dt.float32
    bf16 = mybir.dt.bfloat16
    AF = mybir.ActivationFunctionType
    ALU = mybir.AluOpType

    nc = bacc.Bacc("TRN2", target_bir_lowering=False, debug=False,
                   num_devices=N_CORES)
    WB_d = nc.dram_tensor("WB", [QB, 128, J], f32, kind="ExternalInput").ap()
    uB_d = nc.dram_tensor("uB", [128, QB * B], f32, kind="ExternalInput").ap()
    uZ_d = nc.dram_tensor("uZ", [128, 2 * QB * B], f32, kind="ExternalInput").ap()
    v_d = nc.dram_tensor("v_out", [B, J], f32, kind="ExternalOutput").ap()

    with tile.TileContext(nc) as tc:
        with (
            tc.tile_pool(name="wpool", bufs=1) as wpool,
            tc.tile_pool(name="upool", bufs=1) as upool,
            tc.tile_pool(name="state", bufs=1) as state,
            tc.tile_pool(name="scratch", bufs=1) as scratch,
            tc.tile_pool(name="smalls", bufs=4) as smalls,
            tc.tile_pool(name="psum", bufs=2, space="PSUM") as pp,
            tc.tile_pool(name="psacc", bufs=1, space="PSUM") as pacc,
            tc.tile_pool(name="dram", bufs=2, space="DRAM") as dram,
        ):
            # --- load inputs ---
            w_tiles = []
            for q in range(QB):
                wt = wpool.tile([128, J], f32, tag=f"w{q}")
                nc.sync.dma_start(wt[:], WB_d[q])
                w_tiles.append(wt)
            uB_t = upool.tile([128, QB * B], f32, tag="uB")
            nc.sync.dma_start(uB_t[:], uB_d[:])
            uZ_t = upool.tile([128, 2 * QB * B], f32, tag="uZ")
            nc.sync.dma_start(uZ_t[:], uZ_d[:])

            blog = state.tile([B, NL * N_OUT], f32, tag="blog")  # logits
            nc.vector.memset(blog[:], 0.0)
            v_t = state.tile([B, J], f32, tag="v")

            def allreduce_squash(ps_s, scale):
                """PSUM s-partial -> AllReduce -> squash -> v_t."""
                s_loc = scratch.tile([B, J], f32, tag="st")
                nc.scalar.mul(s_loc[:], ps_s[:], scale)
                bin_ = dram.tile([B, J], f32, tag="bounce_in")
                bout = dram.tile([B, J], f32, tag="bounce_out")
                nc.sync.dma_start(bin_[:], s_loc[:])
                nc.gpsimd.collective_compute(
                    "AllReduce", ALU.add,
                    replica_groups=[list(range(N_CORES))],
                    ins=[bin_.opt()], outs=[bout.opt()],
                )
                s_g = scratch.tile([B, J], f32, tag="st2")
                nc.sync.dma_start(s_g[:], bout[:])
                # squash: scale_o = sqrt(n2)/(1+n2)  (= n2/(1+n2)/sqrt(n2))
                sq = scratch.tile([B, J], f32, tag="t2")
                nc.vector.tensor_mul(sq[:], s_g[:], s_g[:])
                n2 = smalls.tile([B, N_OUT], f32, tag="n2")
                nc.vector.reduce_sum(
                    n2[:], sq[:].rearrange("p (o k) -> p o k", k=D_OUT),
                    axis=mybir.AxisListType.X)
                n2p1 = smalls.tile([B, N_OUT], f32, tag="n2p1")
                nc.scalar.add(n2p1[:], n2[:], 1.0)
                rcp = smalls.tile([B, N_OUT], f32, tag="rcp")
                nc.vector.reciprocal(rcp[:], n2p1[:])
                rt = smalls.tile([B, N_OUT], f32, tag="rt")
                nc.scalar.activation(rt[:], n2[:], AF.Sqrt)
                scl = smalls.tile([B, N_OUT], f32, tag="scl")
                nc.vector.tensor_mul(scl[:], rt[:], rcp[:])
                nc.vector.tensor_mul(
                    v_t[:].rearrange("p (o k) -> p o k", k=D_OUT),
                    s_g[:].rearrange("p (o k) -> p o k", k=D_OUT),
                    scl[:].unsqueeze(2).broadcast_to([B, N_OUT, D_OUT]))
                return s_g

            # ---- pass A: s1 = (1/32) * sum_n uhat ----
            psA = pacc.tile([B, J], f32, tag="ps_s")
            for q in range(QB):
                for jh in range(2):
                    nc.tensor.matmul(
                        psA[:, jh * 512:(jh + 1) * 512],
                        lhsT=uB_t[:, q * B:(q + 1) * B],
                        rhs=w_tiles[q][:, jh * 512:(jh + 1) * 512],
                        start=(q == 0), stop=(q == QB - 1))
            allreduce_squash(psA, 1.0 / N_OUT)

            # ---- passes B, C ----
            for it in range(2):
                ps_s = pacc.tile([B, J], f32, tag="ps_s")
                for n in range(NL):
                    q, p8 = divmod(n, 8)
                    r, h = divmod(p8, 2)
                    psU = pp.tile([B, J], f32, tag="uhat")
                    for jh in range(2):
                        nc.tensor.matmul(
                            psU[:, jh * 512:(jh + 1) * 512],
                            lhsT=uZ_t[32 * r:32 * r + 32,
                                      (2 * q + h) * B:(2 * q + h + 1) * B],
                            rhs=w_tiles[q][32 * r:32 * r + 32,
                                           jh * 512:(jh + 1) * 512],
                            start=True, stop=True,
                            tile_position=(32 * r, 0))
                    # a_n[b,o] = sum_k uhat*v
                    tmp = pp.tile([B, J], f32, tag="scr", bufs=1)
                    nc.vector.tensor_mul(tmp[:], psU[:], v_t[:])
                    a_n = smalls.tile([B, N_OUT], f32, tag="a_n")
                    nc.vector.reduce_sum(
                        a_n[:], tmp[:].rearrange("p (o k) -> p o k", k=D_OUT),
                        axis=mybir.AxisListType.X)
                    blog_n = blog[:, n * N_OUT:(n + 1) * N_OUT]
                    nc.vector.tensor_add(blog_n, blog_n, a_n[:])
                    # c_n = softmax(blog_n) over o
                    mx = smalls.tile([B, 1], f32, tag="mx")
                    nc.vector.reduce_max(mx[:], blog_n, axis=mybir.AxisListType.X)
                    nmx = smalls.tile([B, 1], f32, tag="nmx")
                    nc.scalar.mul(nmx[:], mx[:], -1.0)
                    e = smalls.tile([B, N_OUT], f32, tag="e")
                    nc.scalar.activation(e[:], blog_n, AF.Exp, bias=nmx[:])
                    sm = smalls.tile([B, 1], f32, tag="sm")
                    nc.vector.reduce_sum(sm[:], e[:], axis=mybir.AxisListType.X)
                    rc = smalls.tile([B, 1], f32, tag="rc")
                    nc.vector.reciprocal(rc[:], sm[:])
                    c_n = smalls.tile([B, N_OUT], f32, tag="c_n")
                    nc.vector.tensor_scalar_mul(c_n[:], e[:], rc[:])
                    # s_acc += c_n (bcast k) * uhat
                    t2 = scratch.tile([B, J], f32, tag="t2")
                    nc.vector.tensor_mul(
                        t2[:].rearrange("p (o k) -> p o k", k=D_OUT),
                        psU[:].rearrange("p (o k) -> p o k", k=D_OUT),
                        c_n[:].unsqueeze(2).broadcast_to([B, N_OUT, D_OUT]))
                    if n == 0:
                        nc.vector.tensor_copy(ps_s[:], t2[:])
                    else:
                        nc.vector.tensor_add(ps_s[:], ps_s[:], t2[:])
                allreduce_squash(ps_s, 1.0)

            nc.sync.dma_start(v_d[:], v_t[:])

    nc.compile()
    return nc


def _get_program():
    if "nc" not in _CACHE:
        _CACHE["nc"] = _build_program()
    return _CACHE["nc"]


def kernel(u, W):
    from concourse.bass_utils import run_bass_kernel_spmd

    nc = _get_program()
    in_maps = _pack_inputs(np.asarray(u, np.float32), np.asarray(W, np.float32))
    res = run_bass_kernel_spmd(nc, in_maps, list(range(N_CORES)))
    v = res.results[0]["v_out"]
    return v.reshape(B, N_OUT, D_OUT)


# revision 6
# speedup vs baseline: 1.5347x; 1.5347x over previous
"""Trainium2 Bass kernel for capsule-network dynamic routing.

Problem: u [64, 2048, 16], W [2048, 16, 1024] ->
  uhat = einsum('bni,nij->bnj', u, W)  (viewed [B, N, 32, 32])
  3 routing iterations (softmax over out-caps, squash) -> v [64, 32, 32]

Sharding: n (input capsules) split across 8 cores, 256 per core.
W slice stays SBUF-resident; uhat is recomputed on the PE each routing
pass (never materialized to HBM).  The per-iteration s-reduction
([64, 32, 32] partial sums) is AllReduced across cores.

Per-core n indexing: n = q*8 + r*2 + h  (q: 32 W-blocks, r: 4 PE row
groups, h: 2 PSUM column groups).  uhat for 2 capsules (h = 0, 1) is
stacked on PSUM partitions [128 = 64h + b] so every VE op runs with all
128 lanes busy.

Host-side layouts per core:
  WB [32, 128, 1024]: WB[q, 16*p8+i, j] = W[q*8+p8, i, j]
  uB [128, 2048]:     uB[16*p8+i, q*64+b] = u[b, q*8+p8, i]   (pass A)
  uZ [128, 4096]:     uZ[32r+16h+i, (2q+h)*64+b] = u[b, n, i], 0 elsewhere
  I2 [128, 64]:       vertically stacked 64x64 identities (h-merge)
"""

import numpy as np

B = 64
N_FULL = 2048
D_IN = 16
N_OUT = 32
D_OUT = 32
J = N_OUT * D_OUT  # 1024
N_CORES = 8
NL = N_FULL // N_CORES  # 256 local capsules
QB = NL // 8  # 32 q-blocks
RH = 2  # r's per half-chunk

_CACHE = {}


def _pack_inputs(u, W):
    """Shard along n and build per-core SBUF-friendly layouts."""
    I2 = np.tile(np.eye(B, dtype=np.float32), (2, 1))
    in_maps = []
    for c in range(N_CORES):
        ul = u[:, c * NL:(c + 1) * NL, :]          # [64, 256, 16]
        Wl = W[c * NL:(c + 1) * NL]                # [256, 16, 1024]
        WB = np.ascontiguousarray(
            Wl.reshape(QB, 8, D_IN, J).reshape(QB, 128, J))
        uB = np.ascontiguousarray(
            ul.reshape(B, QB, 8, D_IN).transpose(2, 3, 1, 0).reshape(128, QB * B))
        uZ = np.zeros((4, 32, 2 * QB, B), dtype=np.float32)  # [r, row32, col/64, b]
        un = ul.reshape(B, QB, 4, 2, D_IN)  # [b, q, r, h, i]
        for h in range(2):
            uZ[:, 16 * h:16 * h + 16, h::2, :] = un[:, :, :, h, :].transpose(2, 3, 1, 0)
        uZ = uZ.reshape(4 * 32, 2 * QB * B)
        in_maps.append({"WB": WB, "uB": uB, "uZ": uZ, "I2": I2})
    return in_maps


def _build_program():
    import concourse.bass as bass
    import concourse.tile as tile
    from concourse import bacc, mybir

    f32 = mybir.dt.float32
    AF = mybir.ActivationFunctionType
    ALU = mybir.AluOpType
    AX = mybir.AxisListType

    nc = bacc.Bacc("TRN2", target_bir_lowering=False, debug=False,
                   num_devices=N_CORES)
    WB_d = nc.dram_tensor("WB", [QB, 128, J], f32, kind="ExternalInput").ap()
    uB_d = nc.dram_tensor("uB", [128, QB * B], f32, kind="ExternalInput").ap()
    uZ_d = nc.dram_tensor("uZ", [128, 2 * QB * B], f32, kind="ExternalInput").ap()
    I2_d = nc.dram_tensor("I2", [128, B], f32, kind="ExternalInput").ap()
    v_d = nc.dram_tensor("v_out", [B, J], f32, kind="ExternalOutput").ap()

    with tile.TileContext(nc) as tc:
        with (
            tc.tile_pool(name="wpool", bufs=1) as wpool,
            tc.tile_pool(name="upool", bufs=1) as upool,
            tc.tile_pool(name="state", bufs=1) as state,
            tc.tile_pool(name="scratch", bufs=1) as scratch,
            tc.tile_pool(name="smalls", bufs=2) as smalls,
            tc.tile_pool(name="psum", bufs=2, space="PSUM") as pp,
            tc.tile_pool(name="dram", bufs=2, space="DRAM") as dram,
        ):
            # --- load inputs ---
            w_tiles = []
            for q in range(QB):
                wt = wpool.tile([128, J], f32, tag=f"w{q}")
                nc.sync.dma_start(wt[:], WB_d[q])
                w_tiles.append(wt)
            uB_t = upool.tile([128, QB * B], f32, tag="uB")
            nc.sync.dma_start(uB_t[:], uB_d[:])
            uZ_t = upool.tile([128, 2 * QB * B], f32, tag="uZ")
            nc.sync.dma_start(uZ_t[:], uZ_d[:])
            I2_t = upool.tile([128, B], f32, tag="I2")
            nc.sync.dma_start(I2_t[:], I2_d[:])

            # logits: blog[64h+b, (q*4+r)*32+o] for n = q*8+r*2+h
            blog = state.tile([128, NL // 2 * N_OUT], f32, tag="blog")
            nc.gpsimd.memset(blog[:], 0.0)
            v_t = state.tile([128, J], f32, tag="v")  # v duplicated on halves
            s_acc = state.tile([128, J], f32, tag="s_acc")

            def merge_ar_squash(src_sb, scale, merged):
                """s partial -> (h-merge) -> AllReduce -> squash -> v_t."""
                if merged is None:
                    psM = pp.tile([B, J], f32, tag="uchunk", bufs=2)
                    for jh in range(2):
                        nc.tensor.matmul(
                            psM[:, jh * 512:(jh + 1) * 512],
                            lhsT=I2_t[:],
                            rhs=src_sb[:, jh * 512:(jh + 1) * 512],
                            start=True, stop=True)
                    merged = psM
                s_loc = scratch.tile([B, J], f32, tag="st")
                nc.scalar.mul(s_loc[:], merged[:], scale)
                bin_ = dram.tile([B, J], f32, tag="bounce_in")
                bout = dram.tile([B, J], f32, tag="bounce_out")
                nc.sync.dma_start(bin_[:], s_loc[:])
                nc.gpsimd.collective_compute(
                    "AllReduce", ALU.add,
                    replica_groups=[list(range(N_CORES))],
                    ins=[bin_.opt()], outs=[bout.opt()],
                )
                s_g = scratch.tile([B, J], f32, tag="st2")
                nc.sync.dma_start(s_g[:], bout[:])
                # squash: v = s * sqrt(n2)/(1+n2)
                sq = scratch.tile([B, J], f32, tag="sq")
                nc.vector.tensor_mul(sq[:], s_g[:], s_g[:])
                n2 = smalls.tile([B, N_OUT], f32, tag="n2")
                nc.vector.reduce_sum(
                    n2[:], sq[:].rearrange("p (o k) -> p o k", k=D_OUT), axis=AX.X)
                n2p1 = smalls.tile([B, N_OUT], f32, tag="n2p1")
                nc.scalar.add(n2p1[:], n2[:], 1.0)
                rcp = smalls.tile([B, N_OUT], f32, tag="rcp")
                nc.vector.reciprocal(rcp[:], n2p1[:])
                rt = smalls.tile([B, N_OUT], f32, tag="rt")
                nc.scalar.activation(rt[:], n2[:], AF.Sqrt)
                scl = smalls.tile([B, N_OUT], f32, tag="scl")
                nc.vector.tensor_mul(scl[:], rt[:], rcp[:])
                nc.vector.tensor_mul(
                    v_t[0:B, :].rearrange("p (o k) -> p o k", k=D_OUT),
                    s_g[:].rearrange("p (o k) -> p o k", k=D_OUT),
                    scl[:].unsqueeze(2).broadcast_to([B, N_OUT, D_OUT]))
                # duplicate onto partitions 64..127 for 128-lane consumers
                nc.sync.dma_start(v_t[B:2 * B, :], v_t[0:B, :])

            # ---- pass A: s1 = (1/32) * sum_n uhat ----
            psA = pp.tile([B, J], f32, tag="uchunk", bufs=2)
            for q in range(QB):
                for jh in range(2):
                    nc.tensor.matmul(
                        psA[:, jh * 512:(jh + 1) * 512],
                        lhsT=uB_t[:, q * B:(q + 1) * B],
                        rhs=w_tiles[q][:, jh * 512:(jh + 1) * 512],
                        start=(q == 0), stop=(q == QB - 1))
            merge_ar_squash(None, 1.0 / N_OUT, psA)

            # ---- passes B, C ----
            # half-chunk: one q, RH r's, both h -> 2*RH capsules,
            # uhat in PSUM [128 = 64h+b partitions, RH*1024].
            for it in range(2):
                nc.gpsimd.memset(s_acc[:], 0.0)
                for q in range(QB):
                    for rr in range(4 // RH):
                        psU = pp.tile([128, RH * J], f32, tag="uchunk", bufs=2)
                        for dr in range(RH):
                            r = rr * RH + dr
                            for h in range(2):
                                for jh in range(2):
                                    nc.tensor.matmul(
                                        psU[B * h:B * (h + 1),
                                            dr * J + jh * 512:dr * J + (jh + 1) * 512],
                                        lhsT=uZ_t[32 * r:32 * r + 32,
                                                  (2 * q + h) * B:(2 * q + h + 1) * B],
                                        rhs=w_tiles[q][32 * r:32 * r + 32,
                                                       jh * 512:(jh + 1) * 512],
                                        start=True, stop=True,
                                        tile_position=(32 * r, B * h))
                        # a[128, RH*32] = sum_k uhat * v
                        tmp = scratch.tile([128, RH * J], f32, tag="tt")
                        nc.vector.tensor_mul(
                            tmp[:], psU[:],
                            v_t[:].unsqueeze(1).broadcast_to([128, RH, J]))
                        aa = smalls.tile([128, RH * N_OUT], f32, tag="aa")
                        nc.vector.reduce_sum(
                            aa[:], tmp[:].rearrange("p (g k) -> p g k", k=D_OUT),
                            axis=AX.X)
                        bslice = blog[:, (q * 4 + rr * RH) * N_OUT:
                                      (q * 4 + rr * RH + RH) * N_OUT]
                        nc.gpsimd.tensor_add(bslice, bslice, aa[:])
                        # softmax over o (last 32) per (partition, r)
                        mx = smalls.tile([128, RH], f32, tag="mx")
                        nc.vector.reduce_max(
                            mx[:], bslice.rearrange("p (g o) -> p g o", o=N_OUT),
                            axis=AX.X)
                        eb = smalls.tile([128, RH * N_OUT], f32, tag="eb")
                        nc.gpsimd.tensor_sub(
                            eb[:].rearrange("p (g o) -> p g o", o=N_OUT),
                            bslice.rearrange("p (g o) -> p g o", o=N_OUT),
                            mx[:].unsqueeze(2).broadcast_to([128, RH, N_OUT]))
                        ee = smalls.tile([128, RH * N_OUT], f32, tag="ee")
                        nc.scalar.activation(ee[:], eb[:], AF.Exp)
                        sm = smalls.tile([128, RH], f32, tag="sm")
                        nc.vector.reduce_sum(
                            sm[:], ee[:].rearrange("p (g o) -> p g o", o=N_OUT),
                            axis=AX.X)
                        rc = smalls.tile([128, RH], f32, tag="rc")
                        nc.vector.reciprocal(rc[:], sm[:])
                        cc = smalls.tile([128, RH * N_OUT], f32, tag="cc")
                        nc.gpsimd.tensor_mul(
                            cc[:].rearrange("p (g o) -> p g o", o=N_OUT),
                            ee[:].rearrange("p (g o) -> p g o", o=N_OUT),
                            rc[:].unsqueeze(2).broadcast_to([128, RH, N_OUT]))
                        # s_acc += sum_r c (bcast k) * uhat
                        t2 = scratch.tile([128, RH * J], f32, tag="tt")
                        nc.vector.tensor_mul(
                            t2[:].rearrange("p (g k) -> p g k", k=D_OUT),
                            psU[:].rearrange("p (g k) -> p g k", k=D_OUT),
                            cc[:].unsqueeze(2).broadcast_to(
                                [128, RH * N_OUT, D_OUT]))
                        for dr in range(RH):
                            nc.gpsimd.tensor_add(
                                s_acc[:], s_acc[:], t2[:, dr * J:(dr + 1) * J])
                merge_ar_squash(s_acc, 1.0, None)

            nc.sync.dma_start(v_d[:], v_t[0:B, :])

    nc.compile()
    return nc


def _get_program():
    if "nc" not in _CACHE:
        _CACHE["nc"] = _build_program()
    return _CACHE["nc"]


def kernel(u, W):
    from concourse.bass_utils import run_bass_kernel_spmd

    nc = _get_program()
    in_maps = _pack_inputs(np.asarray(u, np.float32), np.asarray(W, np.float32))
    res = run_bass_kernel_spmd(nc, in_maps, list(range(N_CORES)))
    v = res.results[0]["v_out"]
    return v.reshape(B, N_OUT, D_OUT)


# revision 7
# speedup vs baseline: 1.6909x; 1.1018x over previous
"""Trainium2 Bass kernel for capsule-network dynamic routing.

Problem: u [64, 2048, 16], W [2048, 16, 1024] ->
  uhat = einsum('bni,nij->bnj', u, W)  (viewed [B, N, 32, 32])
  3 routing iterations (softmax over out-caps, squash) -> v [64, 32, 32]

Sharding: n (input capsules) split across 8 cores, 256 per core.
W slice stays SBUF-resident; uhat is recomputed on the PE each routing
pass (never materialized to HBM).  The per-iteration s-reduction
([64, 32, 32] partial sums) is AllReduced across cores.

Per-core n indexing: n = q*8 + r*2 + h  (q: 32 W-blocks, r: 4 PE row
groups, h: 2 PSUM column groups).  A "chunk" is one (q, r): its two
capsules (h = 0, 1) are stacked on PSUM partitions [128 = 64h + b] so
every VE/GpSimd op runs with all 128 lanes busy.

Pipeline per chunk (engines overlapped across chunks):
  PE:     4 matmuls -> psU [128, 1024] fp32 (uhat pair)
  ACT:    evacuate psU -> uh bf16 (also: Exp for softmax)
  VE:     tmp = uh * v (bf16 2x); a = reduce_k(tmp); softmax reduces
  GpSimd: logits += a; softmax elementwise; ccx = c expanded over k
  VE:     t2 = uh * ccx (bf16 2x)
  PE:     psS += I2b^T @ t2   (s-accumulation, K=128 identity-stack)

Host-side layouts per core:
  WB [32, 128, 1024]: WB[q, 16*p8+i, j] = W[q*8+p8, i, j]
  uB [128, 2048]:     uB[16*p8+i, q*64+b] = u[b, q*8+p8, i]   (pass A)
  uZ [128, 4096]:     uZ[32r+16h+i, (2q+h)*64+b] = u[b, n, i], 0 elsewhere
  I2B [128, 64]:      stacked 64x64 identities, bf16 (h/b-merge)
"""

import numpy as np

B = 64
N_FULL = 2048
D_IN = 16
N_OUT = 32
D_OUT = 32
J = N_OUT * D_OUT  # 1024
N_CORES = 8
NL = N_FULL // N_CORES  # 256 local capsules
QB = NL // 8  # 32 q-blocks

_CACHE = {}


def _pack_inputs(u, W):
    """Shard along n and build per-core SBUF-friendly layouts."""
    import ml_dtypes
    I2B = np.tile(np.eye(B, dtype=np.float32), (2, 1)).astype(ml_dtypes.bfloat16)
    in_maps = []
    for c in range(N_CORES):
        ul = u[:, c * NL:(c + 1) * NL, :]          # [64, 256, 16]
        Wl = W[c * NL:(c + 1) * NL]                # [256, 16, 1024]
        WB = np.ascontiguousarray(
            Wl.reshape(QB, 8, D_IN, J).reshape(QB, 128, J))
        uB = np.ascontiguousarray(
            ul.reshape(B, QB, 8, D_IN).transpose(2, 3, 1, 0).reshape(128, QB * B))
        uZ = np.zeros((4, 32, 2 * QB, B), dtype=np.float32)
        un = ul.reshape(B, QB, 4, 2, D_IN)  # [b, q, r, h, i]
        for h in range(2):
            uZ[:, 16 * h:16 * h + 16, h::2, :] = un[:, :, :, h, :].transpose(2, 3, 1, 0)
        uZ = uZ.reshape(4 * 32, 2 * QB * B)
        in_maps.append({"WB": WB, "uB": uB, "uZ": uZ, "I2B": I2B})
    return in_maps


def _build_program():
    import concourse.bass as bass
    import concourse.tile as tile
    from concourse import bacc, mybir

    f32 = mybir.dt.float32
    bf16 = mybir.dt.bfloat16
    AF = mybir.ActivationFunctionType
    ALU = mybir.AluOpType
    AX = mybir.AxisListType

    nc = bacc.Bacc("TRN2", target_bir_lowering=False, debug=False,
                   num_devices=N_CORES)
    WB_d = nc.dram_tensor("WB", [QB, 128, J], f32, kind="ExternalInput").ap()
    uB_d = nc.dram_tensor("uB", [128, QB * B], f32, kind="ExternalInput").ap()
    uZ_d = nc.dram_tensor("uZ", [128, 2 * QB * B], f32, kind="ExternalInput").ap()
    I2B_d = nc.dram_tensor("I2B", [128, B], bf16, kind="ExternalInput").ap()
    v_d = nc.dram_tensor("v_out", [B, J], f32, kind="ExternalOutput").ap()

    with tile.TileContext(nc) as tc:
        with (
            tc.tile_pool(name="wpool", bufs=1) as wpool,
            tc.tile_pool(name="state", bufs=1) as state,
            tc.tile_pool(name="scratch", bufs=2) as scratch,
            tc.tile_pool(name="smalls", bufs=2) as smalls,
            tc.tile_pool(name="psum", bufs=2, space="PSUM") as pp,
            tc.tile_pool(name="dram", bufs=2, space="DRAM") as dram,
        ):
            # --- load inputs ---
            w_tiles = []
            for q in range(QB):
                wt = wpool.tile([128, J], f32, tag=f"w{q}")
                nc.sync.dma_start(wt[:], WB_d[q])
                w_tiles.append(wt)
            uB_t = state.tile([128, QB * B], f32, tag="uB")
            nc.sync.dma_start(uB_t[:], uB_d[:])
            uZ_t = state.tile([128, 2 * QB * B], f32, tag="uZ")
            nc.sync.dma_start(uZ_t[:], uZ_d[:])
            I2B_t = state.tile([128, B], bf16, tag="I2B")
            nc.sync.dma_start(I2B_t[:], I2B_d[:])

            # logits: blog[64h+b, (q*4+r)*32+o] for n = q*8+r*2+h
            blog = state.tile([128, NL // 2 * N_OUT], f32, tag="blog")
            nc.gpsimd.memset(blog[:], 0.0)
            v_t = state.tile([B, J], f32, tag="v")
            v_bf = state.tile([128, J], bf16, tag="v_bf")

            def ar_squash(merged_ps, scale):
                """merged [64,J] psum -> AllReduce -> squash -> v_t, v_bf."""
                s_loc = scratch.tile([B, J], f32, tag="st")
                nc.scalar.mul(s_loc[:], merged_ps[:], scale)
                bin_ = dram.tile([B, J], f32, tag="bounce_in")
                bout = dram.tile([B, J], f32, tag="bounce_out")
                nc.sync.dma_start(bin_[:], s_loc[:])
                nc.gpsimd.collective_compute(
                    "AllReduce", ALU.add,
                    replica_groups=[list(range(N_CORES))],
                    ins=[bin_.opt()], outs=[bout.opt()],
                )
                s_g = scratch.tile([B, J], f32, tag="st2")
                nc.sync.dma_start(s_g[:], bout[:])
                # squash: v = s * sqrt(n2)/(1+n2)
                sq = scratch.tile([B, J], f32, tag="st")
                nc.vector.tensor_mul(sq[:], s_g[:], s_g[:])
                n2 = smalls.tile([B, N_OUT], f32, tag="n2")
                nc.vector.reduce_sum(
                    n2[:], sq[:].rearrange("p (o k) -> p o k", k=D_OUT), axis=AX.X)
                n2p1 = smalls.tile([B, N_OUT], f32, tag="n2p1")
                nc.scalar.add(n2p1[:], n2[:], 1.0)
                rcp = smalls.tile([B, N_OUT], f32, tag="rcp")
                nc.vector.reciprocal(rcp[:], n2p1[:])
                rt = smalls.tile([B, N_OUT], f32, tag="rt")
                nc.scalar.activation(rt[:], n2[:], AF.Sqrt)
                scl = smalls.tile([B, N_OUT], f32, tag="scl")
                nc.vector.tensor_mul(scl[:], rt[:], rcp[:])
                nc.vector.tensor_mul(
                    v_t[:].rearrange("p (o k) -> p o k", k=D_OUT),
                    s_g[:].rearrange("p (o k) -> p o k", k=D_OUT),
                    scl[:].unsqueeze(2).broadcast_to([B, N_OUT, D_OUT]))
                nc.vector.tensor_copy(v_bf[0:B, :], v_t[:])
                nc.sync.dma_start(v_bf[B:2 * B, :], v_bf[0:B, :])

            # ---- pass A: s1 = (1/32) * sum_n uhat ----
            psA = pp.tile([B, J], f32, tag="uchunk", bufs=2)
            for q in range(QB):
                for jh in range(2):
                    nc.tensor.matmul(
                        psA[:, jh * 512:(jh + 1) * 512],
                        lhsT=uB_t[:, q * B:(q + 1) * B],
                        rhs=w_tiles[q][:, jh * 512:(jh + 1) * 512],
                        start=(q == 0), stop=(q == QB - 1))
            ar_squash(psA, 1.0 / N_OUT)

            # ---- passes B, C ----
            for it in range(2):
                psS = pp.tile([B, J], f32, tag="psS", bufs=1)
                for q in range(QB):
                    for r in range(4):
                        first = (q == 0 and r == 0)
                        last = (q == QB - 1 and r == 3)
                        ch = q * 4 + r
                        psU = pp.tile([128, J], f32, tag="uchunk", bufs=2)
                        for h in range(2):
                            for jh in range(2):
                                nc.tensor.matmul(
                                    psU[B * h:B * (h + 1),
                                        jh * 512:(jh + 1) * 512],
                                    lhsT=uZ_t[32 * r:32 * r + 32,
                                              (2 * q + h) * B:(2 * q + h + 1) * B],
                                    rhs=w_tiles[q][32 * r:32 * r + 32,
                                                   jh * 512:(jh + 1) * 512],
                                    start=True, stop=True,
                                    tile_position=(32 * r, B * h))
                        # evacuate as bf16 (ScalarE, near PSUM)
                        uh = scratch.tile([128, J], bf16, tag="uh")
                        nc.scalar.mul(uh[:], psU[:], 1.0)
                        # a[128, 32] = sum_k uhat * v   (bf16 2x mult)
                        tmp = scratch.tile([128, J], bf16, tag="tmp", bufs=1)
                        nc.vector.tensor_mul(tmp[:], uh[:], v_bf[:])
                        aa = smalls.tile([128, N_OUT], f32, tag="aa")
                        nc.vector.reduce_sum(
                            aa[:], tmp[:].rearrange("p (o k) -> p o k", k=D_OUT),
                            axis=AX.X)
                        bsl = blog[:, ch * N_OUT:(ch + 1) * N_OUT]
                        nc.gpsimd.tensor_add(bsl, bsl, aa[:])
                        # softmax over o
                        mx = smalls.tile([128, 1], f32, tag="mx")
                        nc.vector.reduce_max(mx[:], bsl, axis=AX.X)
                        eb = smalls.tile([128, N_OUT], f32, tag="eb")
                        nc.gpsimd.tensor_sub(
                            eb[:], bsl, mx[:].broadcast_to([128, N_OUT]))
                        ee = smalls.tile([128, N_OUT], f32, tag="ee")
                        nc.scalar.activation(ee[:], eb[:], AF.Exp)
                        sm = smalls.tile([128, 1], f32, tag="sm")
                        nc.vector.reduce_sum(sm[:], ee[:], axis=AX.X)
                        rc = smalls.tile([128, 1], f32, tag="rc")
                        nc.vector.reciprocal(rc[:], sm[:])
                        # ccx[128, 1024] bf16 = softmax(c), expanded over k
                        ccx = scratch.tile([128, J], bf16, tag="ccx", bufs=1)
                        nc.gpsimd.tensor_mul(
                            ccx[:].rearrange("p (o k) -> p o k", k=D_OUT),
                            ee[:].unsqueeze(2).broadcast_to([128, N_OUT, D_OUT]),
                            rc[:].unsqueeze(2).broadcast_to([128, N_OUT, D_OUT]))
                        # t2 = c * uhat (bf16 2x), then PE folds over (h, b)
                        t2 = scratch.tile([128, J], bf16, tag="t2")
                        nc.vector.tensor_mul(t2[:], uh[:], ccx[:])
                        for jh in range(2):
                            nc.tensor.matmul(
                                psS[:, jh * 512:(jh + 1) * 512],
                                lhsT=I2B_t[:],
                                rhs=t2[:, jh * 512:(jh + 1) * 512],
                                start=first, stop=last)
                ar_squash(psS, 1.0)

            nc.sync.dma_start(v_d[:], v_t[:])

    nc.compile()
    return nc


def _get_program():
    if "nc" not in _CACHE:
        _CACHE["nc"] = _build_program()
    return _CACHE["nc"]


def kernel(u, W):
    from concourse.bass_utils import run_bass_kernel_spmd

    nc = _get_program()
    in_maps = _pack_inputs(np.asarray(u, np.float32), np.asarray(W, np.float32))
    res = run_bass_kernel_spmd(nc, in_maps, list(range(N_CORES)))
    v = res.results[0]["v_out"]
    return v.reshape(B, N_OUT, D_OUT)
